# revision 1
# baseline (speedup 1.0000x reference)
"""Batch MMD loss on 8 Trainium2 NeuronCores.

Reference math per batch (X, Y: [1024, 128]):
    Z = concat(X, Y)                       # [2048, 128]
    D2_ij = |z_i - z_j|^2
    bw = sum(D2) / (n^2 - n)               # detached bandwidth heuristic
    K = exp(-D2 / bw)
    loss_b = mean(K_XX) - 2 mean(K_XY) + mean(K_YY)
output = sum_b loss_b  (32 batches)

Kernel factorization (per batch; each core handles 4 batches):
    u = 1/bw = (n^2-n) / (2*(n*S - |s|^2)),  S = sum_i |z_i|^2, s = sum_i z_i
    K_ij = a_i a_j exp(2u <z_i,z_j>),        a_i = exp(-u |z_i|^2)
    loss_b = (1/N^2) v^T E v,  v_i = sign_i a_i,  E = exp(2u Z Z^T)

E is symmetric, so only a circulant half of the 16x16 grid of 128x128 tiles
is computed: row-tile `it` owns column tiles at circular distance 1..8
(1..7 for it>=8), counted twice, plus a separate x1 diagonal pass. Gram
tiles come from the PE (bf16), exp from ACT (per-partition 2u scale
operand), the i-side weighted reduction from PE matmuls with [128,1]
weight columns accumulating into one packed PSUM bank (strip js at
partition 32*js), and the j-side weights are assembled into the same
packed layout by selector matmuls. A final elementwise multiply + reduce
collapses to one scalar per core; host sums the 8 scalars.

Data parallel: batch dim 32 -> 4 per core across 8 cores.
"""

from contextlib import ExitStack

import numpy as np

import bass_rust
import concourse.bass as bass
import concourse.tile as tile
from concourse import mybir
from concourse.masks import make_identity
from concourse.bass_utils import run_bass_kernel_spmd

FP32 = mybir.dt.float32
BF16 = mybir.dt.bfloat16
AF = mybir.ActivationFunctionType
ALU = mybir.AluOpType

B, N, D = 32, 1024, 128
NCORES = 8
BPC = B // NCORES          # batches per core
n2 = 2 * N                 # 2048 rows in Z
NT = n2 // 128             # 16 row tiles
TS = 128                   # tile size
INV_N2 = 1.0 / (N * N)     # 2^-20, exact in fp32


def _split_multi_waits(nc):
    """The walrus build in this container allows a single sync-wait per
    instruction ("Too many sync wait commands"), while Tile freely attaches
    several. Hoist all but one wait of each instruction onto single-wait
    no-ops inserted just before it on the same engine."""
    nid = [0]
    for f in nc.m.functions:
        for bb in f.blocks:
            insts = bb.instructions
            out = []
            changed = False
            for i in insts:
                si = getattr(i, "sync_info", None)
                if si is not None and len(si.on_wait) > 1:
                    waits = list(si.on_wait)
                    for w in waits[:-1]:
                        nid[0] += 1
                        nop = mybir.InstNoOp(
                            name=f"I-waitsplit-{nid[0]}", ins=[], outs=[]
                        )
                        nop.engine = i.engine
                        nop.sync_info = bass_rust.SyncInfo(
                            on_wait=[w], on_update=[]
                        )
                        out.append(nop)
                    si.on_wait = [waits[-1]]
                    changed = True
                out.append(i)
            if changed:
                bb.instructions = out


def _owned(it):
    """(j0, width) of the circulant off-diagonal strip owned by row-tile it."""
    k = 8 if it < 8 else 7
    return (TS * (it + 1)) % n2, k * TS


def _segments(j0, w):
    """Absolute column segments (split at the 2048 wrap)."""
    if j0 + w <= n2:
        return [(j0, j0 + w)]
    return [(j0, n2), (0, j0 + w - n2)]


def build(debug=False):
    nc = bass.Bass(num_swdge_queues=4)
    x = nc.dram_tensor("x", [BPC, N, D], FP32, kind="ExternalInput")
    y = nc.dram_tensor("y", [BPC, N, D], FP32, kind="ExternalInput")
    out = nc.dram_tensor("out", [1, 1], FP32, kind="ExternalOutput")
    if debug:
        d_r1 = nc.dram_tensor("d_r1", [128, 16], FP32, kind="ExternalOutput")
        d_acol = nc.dram_tensor("d_acol", [128, 16], FP32, kind="ExternalOutput")
        d_u = nc.dram_tensor("d_u", [1, 1], FP32, kind="ExternalOutput")

    with tile.TileContext(nc) as tc, ExitStack() as ctx:
        consts = ctx.enter_context(tc.tile_pool(name="consts", bufs=1))
        zb_p = ctx.enter_context(tc.tile_pool(name="zb", bufs=2))
        zt_p = ctx.enter_context(tc.tile_pool(name="zt", bufs=2))
        zsq_p = ctx.enter_context(tc.tile_pool(name="zsq", bufs=2))
        sm_p = ctx.enter_context(tc.tile_pool(name="sm", bufs=4))
        e_p = ctx.enter_context(tc.tile_pool(name="e", bufs=20))
        ed_p = ctx.enter_context(tc.tile_pool(name="ed", bufs=2))
        acc_p = ctx.enter_context(tc.tile_pool(name="acc", bufs=1))
        pbig = ctx.enter_context(tc.tile_pool(name="pbig", bufs=2, space="PSUM"))
        pr1 = ctx.enter_context(tc.tile_pool(name="pr1", bufs=1, space="PSUM"))
        psm = ctx.enter_context(tc.tile_pool(name="psm", bufs=2, space="PSUM"))

        # --- constants (once per core) ---
        ident = consts.tile([128, 128], FP32)
        make_identity(nc, ident)
        ones_col = consts.tile([128, 1], FP32)
        nc.gpsimd.memset(ones_col, 1.0)
        ones_row = consts.tile([1, 128], FP32)
        nc.gpsimd.memset(ones_row, 1.0)
        zrow_bf = consts.tile([1, 128], BF16)
        nc.gpsimd.memset(zrow_bf, 0.0)
        ones512_bf = consts.tile([1, 512], BF16)
        nc.gpsimd.memset(ones512_bf, 1.0)
        # sign row: +1 for X tiles (t<8), -1 for Y tiles
        sgn16 = consts.tile([128, 16], FP32)
        nc.gpsimd.memset(sgn16[:, 0:8], 1.0)
        nc.gpsimd.memset(sgn16[:, 8:16], -1.0)

        acc = acc_p.tile([1, 1], FP32)

        for b in range(BPC):
            # ---- phase A: load (f32 -> bf16 casting DMA on 4 SWDGE queues) ----
            zb = zb_p.tile([128, NT, D], BF16, tag="zb")
            for half, src in ((0, x), (1, y)):
                s_ap = src.ap()[b].rearrange("(t p) d -> p t d", p=128)
                for c in range(2):
                    nc.gpsimd.dma_start(
                        out=zb[:, half * 8 + c * 4 : half * 8 + (c + 1) * 4, :],
                        in_=s_ap[:, c * 4 : (c + 1) * 4, :],
                    )

            # transposes via DMA xbar (bf16 SBUF->SBUF), Zt[d, i]
            zt = zt_p.tile([128, NT, D], BF16, tag="zt")
            for t in range(NT):
                nc.sync.dma_start(out=zt[:, t, :], in_=zb[:, t, :], transpose=True)
            zt_f = zt[:, :, :].rearrange("p t d -> p (t d)")

            # ---- phase B: stats + bandwidth ----
            zsq = zsq_p.tile([128, NT, D], FP32, tag="zsq")
            nc.vector.tensor_tensor(
                zsq[:, :, :].rearrange("p t d -> p (t d)"),
                zb[:, :, :].rearrange("p t d -> p (t d)"),
                zb[:, :, :].rearrange("p t d -> p (t d)"),
                ALU.mult,
            )
            sq_col = sm_p.tile([128, NT], FP32, tag="sqcol")
            nc.vector.tensor_reduce(
                out=sq_col, in_=zsq, axis=mybir.AxisListType.X, op=ALU.add
            )
            sqsum = sm_p.tile([128, 1], FP32, tag="sqsum")
            nc.vector.tensor_reduce(
                out=sqsum, in_=sq_col, axis=mybir.AxisListType.X, op=ALU.add
            )
            s_sb = sm_p.tile([128, 1], FP32, tag="ssb")
            nc.vector.tensor_reduce(
                out=s_sb, in_=zt, axis=mybir.AxisListType.XY, op=ALU.add
            )

            S_ps = psm.tile([1, 1], FP32, tag="psm")
            nc.tensor.matmul(S_ps, lhsT=sqsum, rhs=ones_col, start=True, stop=True)
            Ssc = sm_p.tile([1, 1], FP32, tag="Ssc")
            nc.scalar.mul(Ssc, S_ps, float(n2))

            T2_ps = psm.tile([1, 1], FP32, tag="psm")
            nc.tensor.matmul(T2_ps, lhsT=s_sb, rhs=s_sb, start=True, stop=True)
            diff = sm_p.tile([1, 1], FP32, tag="diff")
            nc.vector.tensor_tensor(diff, Ssc, T2_ps, ALU.subtract)
            rec = sm_p.tile([1, 1], FP32, tag="rec")
            nc.vector.reciprocal(rec, diff)
            u_sb = sm_p.tile([1, 1], FP32, tag="usb")
            nc.scalar.mul(u_sb, rec, float(n2 * n2 - n2) / 2.0)

            u_ps = psm.tile([128, 1], FP32, tag="psm")
            nc.tensor.matmul(u_ps, lhsT=ones_row, rhs=u_sb, start=True, stop=True)
            scale2u = sm_p.tile([128, 1], FP32, tag="scale2u")
            nc.scalar.mul(scale2u, u_ps, 2.0)
            negu = sm_p.tile([128, 1], FP32, tag="negu")
            nc.scalar.mul(negu, u_ps, -1.0)

            # a_i = exp(-u |z_i|^2); weight columns
            a_col = sm_p.tile([128, NT], FP32, tag="acol")
            nc.scalar.activation(a_col, sq_col, AF.Exp, bias=0.0, scale=negu)
            av_col = sm_p.tile([128, NT], FP32, tag="avcol")
            nc.vector.tensor_tensor(av_col, a_col, sgn16, ALU.mult)
            av2_bf = sm_p.tile([128, NT], BF16, tag="av2bf")
            nc.vector.tensor_scalar_mul(av2_bf, av_col, 2.0)
            avd_bf = sm_p.tile([128, NT], BF16, tag="avdbf")
            nc.vector.tensor_copy(avd_bf, av_col)
            avn_col = sm_p.tile([128, NT], FP32, tag="avncol")
            nc.vector.tensor_scalar_mul(avn_col, av_col, INV_N2)

            # ---- phase C1: E strips (circulant halves) + diagonal tiles ----
            e_tiles = []
            for it in range(NT):
                j0, w = _owned(it)
                p_ps = pbig.tile([128, 1024], FP32, tag="bigP")
                off = 0
                for a, bnd in _segments(j0, w):
                    pos = a
                    while pos < bnd:
                        # each matmul output must stay inside one PSUM bank
                        chunk = min(512 - off % 512, bnd - pos)
                        nc.tensor.matmul(
                            p_ps[:, off : off + chunk],
                            lhsT=zt[:, it, :],
                            rhs=zt_f[:, pos : pos + chunk],
                            start=True,
                            stop=True,
                        )
                        pos += chunk
                        off += chunk
                e_sb = e_p.tile([128, 1024], BF16, tag="E")
                nc.scalar.activation(
                    e_sb[:, 0:w], p_ps[:, 0:w], AF.Exp, bias=0.0, scale=scale2u
                )
                e_tiles.append(e_sb)

            ed_sb = ed_p.tile([128, NT, TS], BF16, tag="Ed")
            for h in range(2):
                pd_ps = pbig.tile([128, 1024], FP32, tag="bigP")
                for q in range(8):
                    jt = h * 8 + q
                    nc.tensor.matmul(
                        pd_ps[:, q * TS : (q + 1) * TS],
                        lhsT=zt[:, jt, :],
                        rhs=zt_f[:, jt * TS : (jt + 1) * TS],
                        start=True,
                        stop=True,
                    )
                nc.scalar.activation(
                    ed_sb[:, h * 8 : (h + 1) * 8, :].rearrange("p t d -> p (t d)"),
                    pd_ps,
                    AF.Exp,
                    bias=0.0,
                    scale=scale2u,
                )

            # ---- phase C2: weighted reduction, E stationary, column output ----
            # R1col[:, jt] = sum over contributing row-tiles it of
            #   E_it[:, local chunk for jt].T @ w_it   (w = 2*sgn*a off-diag, sgn*a diag)
            r1c_ps = pr1.tile([128, NT], FP32, tag="R1C")
            nc.tensor.matmul(
                r1c_ps, lhsT=zrow_bf, rhs=ones512_bf[:, 0:NT],
                start=True, stop=False, skip_group_check=True,
            )
            cmms = []
            for it in range(NT):
                j0, w = _owned(it)
                for c in range(w // TS):
                    jt = ((j0 + c * TS) % n2) // TS
                    cmms.append(
                        (e_tiles[it][:, c * TS : (c + 1) * TS],
                         av2_bf[:, it : it + 1], jt)
                    )
            for jt in range(NT):
                cmms.append((ed_sb[:, jt, :], avd_bf[:, jt : jt + 1], jt))
            for k, (lhsT, rhs, jt) in enumerate(cmms):
                nc.tensor.matmul(
                    r1c_ps[:, jt : jt + 1],
                    lhsT=lhsT,
                    rhs=rhs,
                    start=False,
                    stop=(k == len(cmms) - 1),
                    skip_group_check=True,
                )

            # ---- phase C4: collapse to scalar ----
            r1c_sb = sm_p.tile([128, NT], FP32, tag="r1csb")
            nc.vector.tensor_copy(r1c_sb, r1c_ps)
            q16 = sm_p.tile([128, NT], FP32, tag="q16")
            nc.vector.tensor_tensor(q16, r1c_sb, avn_col, ALU.mult)
            qs = sm_p.tile([128, 1], FP32, tag="qs")
            nc.vector.tensor_reduce(
                out=qs, in_=q16, axis=mybir.AxisListType.X, op=ALU.add
            )
            tb_ps = psm.tile([1, 1], FP32, tag="psm")
            nc.tensor.matmul(tb_ps, lhsT=qs, rhs=ones_col, start=True, stop=True)
            if debug and b == 0:
                nc.gpsimd.dma_start(out=d_r1.ap(), in_=r1c_sb)
                nc.gpsimd.dma_start(out=d_acol.ap(), in_=a_col)
                nc.gpsimd.dma_start(out=d_u.ap(), in_=u_sb)
            if b == 0:
                nc.vector.tensor_copy(acc, tb_ps)
            else:
                nc.vector.tensor_tensor(acc, acc, tb_ps, ALU.add)

        nc.gpsimd.dma_start(out=out.ap(), in_=acc)

    _split_multi_waits(nc)
    return nc


_CACHE = {}


def _get_nc():
    if "nc" not in _CACHE:
        _CACHE["nc"] = build()
    return _CACHE["nc"]


def kernel(allX: np.ndarray, allY: np.ndarray) -> np.ndarray:
    allX = np.ascontiguousarray(allX, dtype=np.float32)
    allY = np.ascontiguousarray(allY, dtype=np.float32)
    nc = _get_nc()
    in_maps = [
        {
            "x": allX[i * BPC : (i + 1) * BPC],
            "y": allY[i * BPC : (i + 1) * BPC],
        }
        for i in range(NCORES)
    ]
    res = run_bass_kernel_spmd(nc, in_maps, core_ids=list(range(NCORES)))
    total = np.float32(0.0)
    for r in res.results:
        total += np.float32(r["out"][0, 0])
    return np.asarray(total, dtype=np.float32)


if __name__ == "__main__":
    rng = np.random.default_rng(0)
    ax = rng.standard_normal((B, N, D)).astype(np.float32)
    ay = rng.standard_normal((B, N, D)).astype(np.float32)
    print(kernel(ax, ay))

